# revision 9
# baseline (speedup 1.0000x reference)
"""Trainium2 Bass kernel for nn_DeepEZGAT (2-layer dense GAT + bias head).

Row-parallel sharding across 8 NeuronCores: each core owns N/8 destination
rows of the NxN attention. Small projection weights are replicated; wh
activations are all-gathered; the [2,60] bias partial is all-reduced.

Softmax uses the identity exp(leaky(s)) = max(exp(s), exp(0.1*s)) with
s = f1[i] + f2[j] separable, so the big [N x N/8] score pass is pure
DVE work (two tensor_scalar muls + max + mask mul) with no max-subtraction
(scores are bounded ~|s|<45, exp stays finite in fp32/bf16).

The attention aggregation contracts over source j on the TensorEngine with
the softmax denominator fused in as a ones-column of the rhs.
"""

import sys

sys.path.insert(0, "/opt/trn_rl_repo")

import numpy as np

import concourse.bass as bass
import concourse.bacc as bacc
import concourse.mybir as mybir
import concourse.tile as tile
from concourse import masks
from concourse.bass_utils import run_bass_kernel_spmd

F32 = mybir.dt.float32
BF16 = mybir.dt.bfloat16
P = 128

FULL_CFG = dict(N=6000, FIN=6000, H=4, O1=30, H2=50, NCORES=8)


def _cdiv(a, b):
    return -(-a // b)


def build_nc(cfg):
    N, FIN, H, O1, H2 = cfg["N"], cfg["FIN"], cfg["H"], cfg["O1"], cfg["H2"]
    NCORES = cfg["NCORES"]
    H1 = H * O1
    R = N // NCORES            # local destination rows per core
    nI = _cdiv(R, P)           # local row blocks
    nJ = _cdiv(N, P)           # source blocks
    nF = _cdiv(FIN, P)         # feature blocks
    Ip = nI * P
    A1 = O1 + 1                # aggregation width per head (+denominator)
    A2 = H2 + 1
    CHUNKB = 24                # column blocks per streamed input chunk

    def blk_chunks(nblocks):
        out = []
        b = 0
        while b < nblocks:
            n = min(CHUNKB, nblocks - b)
            out.append((b, n))
            b += n
        return out

    nc = bacc.Bacc("TRN2", target_bir_lowering=False, debug=False,
                   num_devices=NCORES)

    # ---- external I/O ----
    xs = nc.dram_tensor("xs", [R, FIN], F32, kind="ExternalInput").ap()
    adjs = nc.dram_tensor("adjs", [R, N], F32, kind="ExternalInput").ap()
    w1f = nc.dram_tensor("w1f", [FIN, H1], F32, kind="ExternalInput").ap()
    asrc1r = nc.dram_tensor("asrc1r", [1, H1], F32, kind="ExternalInput").ap()
    adst1r = nc.dram_tensor("adst1r", [1, H1], F32, kind="ExternalInput").ap()
    b1r = nc.dram_tensor("b1r", [1, H1], F32, kind="ExternalInput").ap()
    w2m = nc.dram_tensor("w2m", [H1, H2], F32, kind="ExternalInput").ap()
    asrc2r = nc.dram_tensor("asrc2r", [1, H2], F32, kind="ExternalInput").ap()
    adst2r = nc.dram_tensor("adst2r", [1, H2], F32, kind="ExternalInput").ap()
    b2r = nc.dram_tensor("b2r", [1, H2], F32, kind="ExternalInput").ap()
    wcm = nc.dram_tensor("wcm", [2, H2], F32, kind="ExternalInput").ap()
    ws1s = nc.dram_tensor("ws1s", [60, R], F32, kind="ExternalInput").ap()
    bs1b = nc.dram_tensor("bs1b", [2, 60], F32, kind="ExternalInput").ap()
    ws2b = nc.dram_tensor("ws2b", [2, 60], F32, kind="ExternalInput").ap()
    bs2b = nc.dram_tensor("bs2b", [2, 1], F32, kind="ExternalInput").ap()
    out_logits = nc.dram_tensor("out_logits", [R, 2], F32,
                                kind="ExternalOutput").ap()
    out_bias = nc.dram_tensor("out_bias", [2, 1], F32,
                              kind="ExternalOutput").ap()

    groups = [list(range(NCORES))]

    with tile.TileContext(nc) as tc:
        import contextlib
        ctx = contextlib.ExitStack()
        with ctx:
            dram = ctx.enter_context(tc.tile_pool(name="dram", bufs=1,
                                                  space="DRAM"))
            pconst = ctx.enter_context(tc.tile_pool(name="pconst", bufs=1))
            pbig = ctx.enter_context(tc.tile_pool(name="pbig", bufs=1))
            pwork = ctx.enter_context(tc.tile_pool(name="pwork", bufs=3))

            wh1_loc_d = dram.tile([R, H1], F32)
            wh1_full_d = dram.tile([N, H1], F32)
            wh2_loc_d = dram.tile([R, H2], F32)
            wh2_full_d = dram.tile([N, H2], F32)
            pb_loc_d = dram.tile([2, 60], F32)
            pb_full_d = dram.tile([2, 60], F32)

            # ================= Phase 0: constants =================
            identf = pconst.tile([P, P], F32)
            ident16 = pconst.tile([P, P], BF16)
            masks.make_identity(nc, identf[:])
            masks.make_identity(nc, ident16[:])
            ones = pconst.tile([1, P], F32)
            nc.vector.memset(ones[:], 1.0)

            def load_row(dram_ap, w):
                t = pwork.tile([1, w], F32, tag="rowtmp")
                nc.sync.dma_start(t[:1, :w], dram_ap)
                return t

            def bcast_row(dram_ap, w, pspool):
                """[1,w] DRAM row -> [128,w] SBUF f32 via ones-lhsT matmul."""
                row = load_row(dram_ap, w)
                pbuf = pspool.tile([P, max(w, 1)], F32, tag="bc_ps")
                n0 = 0
                while n0 < w:
                    n1 = min(n0 + 512, w)
                    nc.tensor.matmul(pbuf[:, n0:n1], ones[:1, :P],
                                     row[0:1, n0:n1], start=True, stop=True)
                    n0 = n1
                out = pconst.tile([P, w], F32, tag=f"bc_{dram_ap.tensor.name}")
                nc.vector.tensor_copy(out[:], pbuf[:, :w])
                return out

            with tc.tile_pool(name="ps0", bufs=2, space="PSUM") as ps0:
                adstbc1 = bcast_row(adst1r[:], H1, ps0)
                asrcbc1 = bcast_row(asrc1r[:], H1, ps0)
                b1bc = bcast_row(b1r[:], H1, ps0)
                adstbc2 = bcast_row(adst2r[:], H2, ps0)
                asrcbc2 = bcast_row(asrc2r[:], H2, ps0)
                b2bc = bcast_row(b2r[:], H2, ps0)

                # wcT [H2, 2]
                wcsb = pwork.tile([2, H2], F32, tag="wcsb")
                nc.sync.dma_start(wcsb[:2, :H2], wcm[:])
                pwc = ps0.tile([H2, 2], F32, tag="pwc")
                nc.tensor.transpose(pwc[:H2, :2], wcsb[:2, :H2],
                                    identf[:2, :2])
                wcT = pconst.tile([H2, 2], F32)
                nc.vector.tensor_copy(wcT[:], pwc[:H2, :2])

                # ws1T [R, 60] as nI blocks of [128, 60]
                ws1sb = pconst.tile([60, R], F32)
                nc.sync.dma_start(ws1sb[:60, :R], ws1s[:])
                ws1T = pconst.tile([P, nI * 60], F32)
                for m in range(nI):
                    rows = min(P, R - m * P)
                    pw = ps0.tile([P, 64], F32, tag="pws1")
                    nc.tensor.transpose(pw[:rows, :60],
                                        ws1sb[:60, m * P:m * P + rows],
                                        identf[:60, :60])
                    nc.vector.tensor_copy(
                        ws1T[:rows, m * 60:(m + 1) * 60], pw[:rows, :60])

                w2b = pconst.tile([H1, H2], BF16)
                w2tmp = pwork.tile([H1, H2], F32, tag="w2tmp")
                nc.sync.dma_start(w2tmp[:H1, :H2], w2m[:])
                nc.vector.tensor_copy(w2b[:], w2tmp[:H1, :H2])

                bs1sb = pconst.tile([2, 60], F32)
                nc.sync.dma_start(bs1sb[:2, :60], bs1b[:])
                ws2sb = pconst.tile([2, 60], F32)
                nc.sync.dma_start(ws2sb[:2, :60], ws2b[:])
                bs2sb = pconst.tile([2, 1], F32)
                nc.sync.dma_start(bs2sb[:2, :1], bs2b[:])

            # persistent big tiles
            maskT = pbig.tile([P, nJ * Ip], BF16)
            whaug1 = pbig.tile([P, nJ * H * A1], BF16)
            whaug2 = pbig.tile([P, nJ * A2], BF16)
            f2st = pbig.tile([P, nJ * H], F32)
            ef2st = pbig.tile([P, nJ * H], F32)
            ef201st = pbig.tile([P, nJ * H], F32)
            f2st2 = pbig.tile([P, nJ], F32)
            ef2st2 = pbig.tile([P, nJ], F32)
            ef201st2 = pbig.tile([P, nJ], F32)
            ea1 = pbig.tile([P, H * Ip], BF16)
            eb1 = pbig.tile([P, H * Ip], BF16)
            ea2 = pbig.tile([P, Ip], BF16)
            eb2 = pbig.tile([P, Ip], BF16)
            f1rows = [pbig.tile([1, Ip], F32, name=f"f1rows_{h}",
                                tag=f"f1rows_{h}") for h in range(H)]
            f1rowsT2 = pbig.tile([1, Ip], F32)
            wh1loc = pbig.tile([P, nI * H1], F32)
            wh2loc = pbig.tile([P, nI * H2], F32)
            h1loc = pbig.tile([P, nI * H1], F32)
            h2loc = pbig.tile([P, nI * H2], F32)
            logitsA = pbig.tile([P, nI * 2], F32)

            # ============ Phase 1B: wh1_local = x @ w1 (via PE transpose) ====
            with tc.tile_pool(name="pw1", bufs=1) as pw1pool, \
                 tc.tile_pool(name="pio1", bufs=2) as pio1, \
                 tc.tile_pool(name="ps1", bufs=3, space="PSUM") as ps1:
                w1b = pw1pool.tile([P, nF * H1], BF16)
                for ft in range(nF):
                    frows = min(P, FIN - ft * P)
                    wt = pwork.tile([P, H1], F32, tag="w1tmp")
                    if frows < P:
                        nc.vector.memset(wt[:], 0.0)
                    nc.sync.dma_start(wt[:frows, :H1],
                                      w1f[ft * P:ft * P + frows, :])
                    nc.vector.tensor_copy(w1b[:, ft * H1:(ft + 1) * H1],
                                          wt[:, :H1])

                for m in range(nI):
                    rows = min(P, R - m * P)
                    pacc = ps1.tile([P, H1], F32, tag="whacc")
                    for (b0, nb) in blk_chunks(nF):
                        cw = nb * P
                        c0 = b0 * P
                        creal = min(cw, FIN - c0)
                        xt = pio1.tile([P, CHUNKB * P], F32, tag="io_chunk")
                        if creal < cw:
                            nc.vector.memset(xt[:, :cw], 0.0)
                        nc.sync.dma_start(
                            xt[:rows, :creal],
                            xs[m * P:m * P + rows, c0:c0 + creal])
                        for b in range(nb):
                            ft = b0 + b
                            ptr = ps1.tile([P, P], F32, tag="xtr")
                            nc.tensor.transpose(
                                ptr[:P, :rows],
                                xt[:rows, b * P:(b + 1) * P],
                                identf[:rows, :rows])
                            xTb = pwork.tile([P, P], BF16, tag="xTb")
                            nc.vector.tensor_copy(xTb[:, :rows],
                                                  ptr[:, :rows])
                            nc.tensor.matmul(
                                pacc[:rows, :H1], xTb[:, :rows],
                                w1b[:, ft * H1:(ft + 1) * H1],
                                start=(ft == 0), stop=(ft == nF - 1))
                    nc.vector.tensor_copy(
                        wh1loc[:rows, m * H1:(m + 1) * H1], pacc[:rows, :H1])
                    nc.sync.dma_start(wh1_loc_d[m * P:m * P + rows, :],
                                      wh1loc[:rows, m * H1:(m + 1) * H1])

            nc.gpsimd.collective_compute(
                "AllGather", mybir.AluOpType.bypass, replica_groups=groups,
                ins=[wh1_loc_d.opt()], outs=[wh1_full_d.opt()])

            # ============ Phase 1A: transposed adjacency mask ================
            with tc.tile_pool(name="pio2", bufs=2) as pio2:
                for m in range(nI):
                    rows = min(P, R - m * P)
                    for (b0, nb) in blk_chunks(nJ):
                        cw = nb * P
                        c0 = b0 * P
                        creal = min(cw, N - c0)
                        at = pio2.tile([P, CHUNKB * P], F32, tag="adj_chunk")
                        if creal < cw or rows < P:
                            nc.vector.memset(at[:, :cw], 0.0)
                        nc.sync.dma_start(
                            at[:rows, :creal],
                            adjs[m * P:m * P + rows, c0:c0 + creal])
                        mt = pio2.tile([P, CHUNKB * P], BF16, tag="msk_chunk")
                        nc.vector.tensor_scalar(mt[:, :cw], at[:, :cw], 0.0,
                                                None, mybir.AluOpType.is_gt)
                        for b in range(nb):
                            jt = b0 + b
                            nc.sync.dma_start_transpose(
                                maskT[:, jt * Ip + m * P:jt * Ip + (m + 1) * P],
                                mt[:, b * P:(b + 1) * P])

            # ============ Phase 2: wh1_full prep + f vectors (L1) ============
            with tc.tile_pool(name="ps2", bufs=2, space="PSUM") as ps2:
                for jt in range(nJ):
                    jrows = min(P, N - jt * P)
                    whf = pwork.tile([P, H1], F32, tag="whf")
                    if jrows < P:
                        nc.vector.memset(whf[:], 0.0)
                    nc.sync.dma_start(whf[:jrows, :H1],
                                      wh1_full_d[jt * P:jt * P + jrows, :])
                    base = jt * H * A1
                    for h in range(H):
                        nc.vector.tensor_copy(
                            whaug1[:, base + h * A1:base + h * A1 + O1],
                            whf[:, h * O1:(h + 1) * O1])
                    onecols = whaug1[:, base:base + H * A1].rearrange(
                        "p (h q) -> p h q", q=A1)[:, :, O1:A1]
                    nc.vector.memset(onecols, 1.0)
                    tt = pwork.tile([P, H1], F32, tag="tt")
                    nc.vector.tensor_tensor(tt[:, :H1], whf[:, :H1],
                                            adstbc1[:, :H1],
                                            mybir.AluOpType.mult)
                    nc.vector.tensor_reduce(
                        f2st[:, jt * H:(jt + 1) * H],
                        tt[:, :H1].rearrange("p (h o) -> p h o", o=O1),
                        mybir.AxisListType.X, mybir.AluOpType.add)
                    nc.scalar.activation(ef2st[:, jt * H:(jt + 1) * H],
                                         f2st[:, jt * H:(jt + 1) * H],
                                         mybir.ActivationFunctionType.Exp)
                    nc.scalar.activation(ef201st[:, jt * H:(jt + 1) * H],
                                         f2st[:, jt * H:(jt + 1) * H],
                                         mybir.ActivationFunctionType.Exp,
                                         scale=0.1)

                # f1 rows (local)
                if R < Ip:
                    for h in range(H):
                        nc.vector.memset(f1rows[h][:1, R:Ip], 0.0)
                for m in range(nI):
                    rows = min(P, R - m * P)
                    tt2 = pwork.tile([P, H1], F32, tag="tt")
                    nc.vector.tensor_tensor(
                        tt2[:rows, :H1], wh1loc[:rows, m * H1:(m + 1) * H1],
                        asrcbc1[:rows, :H1], mybir.AluOpType.mult)
                    f1m = pwork.tile([P, H], F32, tag="f1m")
                    nc.vector.tensor_reduce(
                        f1m[:rows, :H],
                        tt2[:rows, :H1].rearrange("p (h o) -> p h o", o=O1),
                        mybir.AxisListType.X, mybir.AluOpType.add)
                    for h in range(H):
                        ptf = ps2.tile([1, P], F32, tag="ptf")
                        nc.tensor.transpose(ptf[:1, :rows],
                                            f1m[:rows, h:h + 1],
                                            identf[:rows, :rows])
                        nc.vector.tensor_copy(
                            f1rows[h][:1, m * P:m * P + rows],
                            ptf[:1, :rows])
                for h in range(H):
                    pb = ps2.tile([P, Ip], F32, tag="pbb")
                    n0 = 0
                    while n0 < Ip:
                        n1 = min(n0 + 512, Ip)
                        nc.tensor.matmul(pb[:, n0:n1], ones[:1, :P],
                                         f1rows[h][0:1, n0:n1],
                                         start=True, stop=True)
                        n0 = n1
                    nc.scalar.activation(ea1[:, h * Ip:(h + 1) * Ip],
                                         pb[:, :Ip],
                                         mybir.ActivationFunctionType.Exp)
                    nc.scalar.activation(eb1[:, h * Ip:(h + 1) * Ip],
                                         pb[:, :Ip],
                                         mybir.ActivationFunctionType.Exp,
                                         scale=0.1)

            # ============ Phase 3: attention layer 1 =========================
            # Head loop outermost so each PSUM bank holds exactly one open
            # accumulation group at a time.
            with tc.tile_pool(name="ps3", bufs=1, space="PSUM") as ps3:
                accs = [ps3.tile([P, A1], F32, name=f"acc1_{m}",
                                 tag=f"acc1_{m}") for m in range(nI)]
                for h in range(H):
                    for jt in range(nJ):
                        u = pwork.tile([P, Ip], BF16, tag="u")
                        v = pwork.tile([P, Ip], BF16, tag="v")
                        pm = pwork.tile([P, Ip], BF16, tag="pm")
                        pp = pwork.tile([P, Ip], BF16, tag="pp")
                        nc.vector.tensor_scalar_mul(
                            u[:], ea1[:, h * Ip:(h + 1) * Ip],
                            ef2st[:, jt * H + h:jt * H + h + 1])
                        nc.vector.tensor_scalar_mul(
                            v[:], eb1[:, h * Ip:(h + 1) * Ip],
                            ef201st[:, jt * H + h:jt * H + h + 1])
                        nc.vector.tensor_tensor(pm[:], u[:], v[:],
                                                mybir.AluOpType.max)
                        nc.vector.tensor_tensor(
                            pp[:], pm[:],
                            maskT[:, jt * Ip:(jt + 1) * Ip],
                            mybir.AluOpType.mult)
                        for m in range(nI):
                            rows = min(P, R - m * P)
                            nc.tensor.matmul(
                                accs[m][:rows, :A1],
                                pp[:, m * P:m * P + rows],
                                whaug1[:, jt * H * A1 + h * A1:
                                       jt * H * A1 + (h + 1) * A1],
                                start=(jt == 0), stop=(jt == nJ - 1))
                    for m in range(nI):
                        rows = min(P, R - m * P)
                        rec = pwork.tile([P, 1], F32, tag="rec")
                        nc.vector.reciprocal(
                            rec[:rows, :], accs[m][:rows, O1:O1 + 1])
                        nc.vector.tensor_scalar_mul(
                            h1loc[:rows, m * H1 + h * O1:m * H1 + (h + 1) * O1],
                            accs[m][:rows, :O1], rec[:rows, :])
                for m in range(nI):
                    rows = min(P, R - m * P)
                    hsl = h1loc[:rows, m * H1:(m + 1) * H1]
                    nc.vector.tensor_tensor(hsl, hsl, b1bc[:rows, :H1],
                                            mybir.AluOpType.add)
                    nc.vector.scalar_tensor_tensor(
                        hsl, hsl, 0.1, hsl,
                        mybir.AluOpType.mult, mybir.AluOpType.max)

            # ============ Phase 4: wh2 + gather + f vectors (L2) =============
            with tc.tile_pool(name="ps4", bufs=1, space="PSUM") as ps4:
                for m in range(nI):
                    rows = min(P, R - m * P)
                    h1b = pwork.tile([P, H1], BF16, tag="h1b")
                    nc.vector.tensor_copy(h1b[:rows, :H1],
                                          h1loc[:rows, m * H1:(m + 1) * H1])
                    pt = ps4.tile([H1, P], BF16, tag="pth")
                    nc.tensor.transpose(pt[:H1, :rows], h1b[:rows, :H1],
                                        ident16[:rows, :rows])
                    hTs = pwork.tile([H1, P], BF16, tag="hTs")
                    nc.vector.tensor_copy(hTs[:H1, :rows], pt[:H1, :rows])
                    pw2 = ps4.tile([P, H2], F32, tag="pw2")
                    nc.tensor.matmul(pw2[:rows, :H2], hTs[:H1, :rows],
                                     w2b[:H1, :H2], start=True, stop=True)
                    nc.vector.tensor_copy(wh2loc[:rows, m * H2:(m + 1) * H2],
                                          pw2[:rows, :H2])
                    nc.sync.dma_start(wh2_loc_d[m * P:m * P + rows, :],
                                      wh2loc[:rows, m * H2:(m + 1) * H2])

                nc.gpsimd.collective_compute(
                    "AllGather", mybir.AluOpType.bypass, replica_groups=groups,
                    ins=[wh2_loc_d.opt()], outs=[wh2_full_d.opt()])

                for jt in range(nJ):
                    jrows = min(P, N - jt * P)
                    whf2 = pwork.tile([P, H2], F32, tag="whf2")
                    if jrows < P:
                        nc.vector.memset(whf2[:], 0.0)
                    nc.sync.dma_start(whf2[:jrows, :H2],
                                      wh2_full_d[jt * P:jt * P + jrows, :])
                    base = jt * A2
                    nc.vector.tensor_copy(whaug2[:, base:base + H2],
                                          whf2[:, :H2])
                    nc.vector.memset(whaug2[:, base + H2:base + A2], 1.0)
                    tt3 = pwork.tile([P, H2], F32, tag="tt3")
                    nc.vector.tensor_tensor(tt3[:, :H2], whf2[:, :H2],
                                            adstbc2[:, :H2],
                                            mybir.AluOpType.mult)
                    nc.vector.tensor_reduce(f2st2[:, jt:jt + 1], tt3[:, :H2],
                                            mybir.AxisListType.X,
                                            mybir.AluOpType.add)
                    nc.scalar.activation(ef2st2[:, jt:jt + 1],
                                         f2st2[:, jt:jt + 1],
                                         mybir.ActivationFunctionType.Exp)
                    nc.scalar.activation(ef201st2[:, jt:jt + 1],
                                         f2st2[:, jt:jt + 1],
                                         mybir.ActivationFunctionType.Exp,
                                         scale=0.1)

                if R < Ip:
                    nc.vector.memset(f1rowsT2[:1, R:Ip], 0.0)
                for m in range(nI):
                    rows = min(P, R - m * P)
                    tt4 = pwork.tile([P, H2], F32, tag="tt3")
                    nc.vector.tensor_tensor(
                        tt4[:rows, :H2], wh2loc[:rows, m * H2:(m + 1) * H2],
                        asrcbc2[:rows, :H2], mybir.AluOpType.mult)
                    f1m2 = pwork.tile([P, 1], F32, tag="f1m2")
                    nc.vector.tensor_reduce(f1m2[:rows, :1], tt4[:rows, :H2],
                                            mybir.AxisListType.X,
                                            mybir.AluOpType.add)
                    ptf2 = ps4.tile([1, P], F32, tag="ptf2")
                    nc.tensor.transpose(ptf2[:1, :rows], f1m2[:rows, :1],
                                        identf[:rows, :rows])
                    nc.vector.tensor_copy(f1rowsT2[:1, m * P:m * P + rows],
                                          ptf2[:1, :rows])
                pb2 = ps4.tile([P, Ip], F32, tag="pbb2")
                n0 = 0
                while n0 < Ip:
                    n1 = min(n0 + 512, Ip)
                    nc.tensor.matmul(pb2[:, n0:n1], ones[:1, :P],
                                     f1rowsT2[0:1, n0:n1],
                                     start=True, stop=True)
                    n0 = n1
                nc.scalar.activation(ea2[:, :Ip], pb2[:, :Ip],
                                     mybir.ActivationFunctionType.Exp)
                nc.scalar.activation(eb2[:, :Ip], pb2[:, :Ip],
                                     mybir.ActivationFunctionType.Exp,
                                     scale=0.1)

            # ============ Phase 5: attention layer 2 =========================
            with tc.tile_pool(name="ps5", bufs=1, space="PSUM") as ps5:
                accs2 = [ps5.tile([P, A2], F32, name=f"acc2_{m}",
                                  tag=f"acc2_{m}") for m in range(nI)]
                for jt in range(nJ):
                    u = pwork.tile([P, Ip], BF16, tag="u")
                    v = pwork.tile([P, Ip], BF16, tag="v")
                    pm = pwork.tile([P, Ip], BF16, tag="pm")
                    pp = pwork.tile([P, Ip], BF16, tag="pp")
                    nc.vector.tensor_scalar_mul(u[:], ea2[:, :Ip],
                                                ef2st2[:, jt:jt + 1])
                    nc.vector.tensor_scalar_mul(v[:], eb2[:, :Ip],
                                                ef201st2[:, jt:jt + 1])
                    nc.vector.tensor_tensor(pm[:], u[:], v[:],
                                            mybir.AluOpType.max)
                    nc.vector.tensor_tensor(pp[:], pm[:],
                                            maskT[:, jt * Ip:(jt + 1) * Ip],
                                            mybir.AluOpType.mult)
                    for m in range(nI):
                        rows = min(P, R - m * P)
                        nc.tensor.matmul(
                            accs2[m][:rows, :A2], pp[:, m * P:m * P + rows],
                            whaug2[:, jt * A2:(jt + 1) * A2],
                            start=(jt == 0), stop=(jt == nJ - 1))
                for m in range(nI):
                    rows = min(P, R - m * P)
                    rec2 = pwork.tile([P, 1], F32, tag="rec")
                    nc.vector.reciprocal(rec2[:rows, :],
                                         accs2[m][:rows, H2:H2 + 1])
                    nc.vector.tensor_scalar_mul(
                        h2loc[:rows, m * H2:(m + 1) * H2],
                        accs2[m][:rows, :H2], rec2[:rows, :])
                    hsl2 = h2loc[:rows, m * H2:(m + 1) * H2]
                    nc.vector.tensor_tensor(hsl2, hsl2, b2bc[:rows, :H2],
                                            mybir.AluOpType.add)
                    nc.vector.scalar_tensor_tensor(
                        hsl2, hsl2, 0.1, hsl2,
                        mybir.AluOpType.mult, mybir.AluOpType.max)

            # ============ Phase 6: classifier head + global bias =============
            with tc.tile_pool(name="ps6", bufs=1, space="PSUM") as ps6:
                ppb = ps6.tile([2, 60], F32, tag="ppb")
                for m in range(nI):
                    rows = min(P, R - m * P)
                    pth2 = ps6.tile([H2, P], F32, tag="pth2")
                    nc.tensor.transpose(pth2[:H2, :rows],
                                        h2loc[:rows, m * H2:(m + 1) * H2],
                                        identf[:rows, :rows])
                    h2T = pwork.tile([H2, P], F32, tag="h2T")
                    nc.vector.tensor_copy(h2T[:H2, :rows], pth2[:H2, :rows])
                    plg = ps6.tile([P, 2], F32, tag="plg")
                    nc.tensor.matmul(plg[:rows, :2], h2T[:H2, :rows],
                                     wcT[:H2, :2], start=True, stop=True)
                    lg0 = pwork.tile([P, 2], F32, tag="lg0")
                    nc.vector.tensor_copy(lg0[:rows, :2], plg[:rows, :2])
                    nc.vector.scalar_tensor_tensor(
                        logitsA[:rows, m * 2:(m + 1) * 2],
                        lg0[:rows, :2], 0.1, lg0[:rows, :2],
                        mybir.AluOpType.mult, mybir.AluOpType.max)
                    nc.tensor.matmul(ppb[:2, :60],
                                     logitsA[:rows, m * 2:(m + 1) * 2],
                                     ws1T[:rows, m * 60:(m + 1) * 60],
                                     start=(m == 0), stop=(m == nI - 1))
                pbloc = pwork.tile([2, 60], F32, tag="pbloc")
                nc.vector.tensor_copy(pbloc[:2, :60], ppb[:2, :60])
                nc.sync.dma_start(pb_loc_d[:], pbloc[:2, :60])
                nc.gpsimd.collective_compute(
                    "AllReduce", mybir.AluOpType.add, replica_groups=groups,
                    ins=[pb_loc_d.opt()], outs=[pb_full_d.opt()])
                pbf = pwork.tile([2, 60], F32, tag="pbf")
                nc.sync.dma_start(pbf[:2, :60], pb_full_d[:])
                bias1 = pwork.tile([2, 60], F32, tag="bias1")
                nc.vector.tensor_tensor(bias1[:2, :60], pbf[:2, :60],
                                        bs1sb[:2, :60], mybir.AluOpType.add)
                nc.vector.scalar_tensor_tensor(
                    bias1[:2, :60], bias1[:2, :60], 0.1, bias1[:2, :60],
                    mybir.AluOpType.mult, mybir.AluOpType.max)
                tws = pwork.tile([2, 60], F32, tag="tws")
                nc.vector.tensor_tensor(tws[:2, :60], bias1[:2, :60],
                                        ws2sb[:2, :60], mybir.AluOpType.mult)
                tb = pwork.tile([2, 1], F32, tag="tb")
                nc.vector.tensor_reduce(tb[:2, :1], tws[:2, :60],
                                        mybir.AxisListType.X,
                                        mybir.AluOpType.add)
                bias2 = pwork.tile([2, 1], F32, tag="bias2")
                nc.vector.tensor_scalar_add(bias2[:2, :1], tb[:2, :1],
                                            bs2sb[:2, :1])
                nc.vector.scalar_tensor_tensor(
                    bias2[:2, :1], bias2[:2, :1], 0.1, bias2[:2, :1],
                    mybir.AluOpType.mult, mybir.AluOpType.max)
                nc.sync.dma_start(out_bias[:], bias2[:2, :1])

                ptb = ps6.tile([1, 2], F32, tag="ptb")
                nc.tensor.transpose(ptb[:1, :2], bias2[:2, :1],
                                    identf[:2, :2])
                b2row = pwork.tile([1, 2], F32, tag="b2row")
                nc.vector.tensor_copy(b2row[:1, :2], ptb[:1, :2])
                pbc = ps6.tile([P, 2], F32, tag="pbc")
                nc.tensor.matmul(pbc[:, :2], ones[:1, :P], b2row[0:1, :2],
                                 start=True, stop=True)
                bcB = pwork.tile([P, 2], F32, tag="bcB")
                nc.vector.tensor_copy(bcB[:], pbc[:, :2])
                for m in range(nI):
                    rows = min(P, R - m * P)
                    outt = pwork.tile([P, 2], F32, tag="outt")
                    nc.vector.tensor_tensor(outt[:rows, :2],
                                            logitsA[:rows, m * 2:(m + 1) * 2],
                                            bcB[:rows, :2],
                                            mybir.AluOpType.add)
                    nc.sync.dma_start(out_logits[m * P:m * P + rows, :],
                                      outt[:rows, :2])

    nc.compile()
    return nc


def make_in_maps(inputs, cfg):
    N, FIN, H, O1, H2 = cfg["N"], cfg["FIN"], cfg["H"], cfg["O1"], cfg["H2"]
    NCORES = cfg["NCORES"]
    R = N // NCORES
    f32 = lambda a: np.ascontiguousarray(a, dtype=np.float32)
    x = f32(inputs["x"])
    adj = f32(inputs["adj"])
    w1 = f32(inputs["w1"])
    common = {
        "w1f": f32(w1.transpose(1, 0, 2).reshape(FIN, H * O1)),
        "asrc1r": f32(inputs["asrc1"].reshape(1, H * O1)),
        "adst1r": f32(inputs["adst1"].reshape(1, H * O1)),
        "b1r": f32(inputs["b1"].reshape(1, H * O1)),
        "w2m": f32(inputs["w2"][0]),
        "asrc2r": f32(inputs["asrc2"].reshape(1, H2)),
        "adst2r": f32(inputs["adst2"].reshape(1, H2)),
        "b2r": f32(inputs["b2"].reshape(1, H2)),
        "wcm": f32(inputs["wc"]),
        "bs1b": f32(np.broadcast_to(inputs["bs1"].reshape(1, 60), (2, 60))),
        "ws2b": f32(np.broadcast_to(inputs["ws2"].reshape(1, 60), (2, 60))),
        "bs2b": f32(np.broadcast_to(inputs["bs2"].reshape(1, 1), (2, 1))),
    }
    ws1 = f32(inputs["ws1"])
    in_maps = []
    for c in range(NCORES):
        m = dict(common)
        m["xs"] = f32(x[c * R:(c + 1) * R])
        m["adjs"] = f32(adj[c * R:(c + 1) * R])
        m["ws1s"] = f32(ws1[:, c * R:(c + 1) * R])
        in_maps.append(m)
    return in_maps


_NC_CACHE = {}


def get_nc(cfg):
    key = tuple(sorted(cfg.items()))
    if key not in _NC_CACHE:
        _NC_CACHE[key] = build_nc(cfg)
    return _NC_CACHE[key]


def assemble(results, cfg):
    N = cfg["N"]
    R = N // cfg["NCORES"]
    logits = np.concatenate([results[c]["out_logits"]
                             for c in range(cfg["NCORES"])], axis=0)
    bias_vec = results[0]["out_bias"][:, 0]
    return (np.ascontiguousarray(logits, dtype=np.float32),
            np.ascontiguousarray(bias_vec, dtype=np.float32))


def kernel(**inputs):
    cfg = FULL_CFG
    nc = get_nc(cfg)
    in_maps = make_in_maps(inputs, cfg)
    res = run_bass_kernel_spmd(nc, in_maps, list(range(cfg["NCORES"])))
    return assemble(res.results, cfg)
